# revision 1
# baseline (speedup 1.0000x reference)
"""Self-contained 2-layer GAT kernel for Trainium2, 8-core SPMD.

Strategy: edges sharded by destination node across the 8 cores (each core
owns a contiguous dst slice; edges sorted by dst tile on host). The node
phase (x@W) is replicated on every core into a bf16 DRAM table (256B rows,
c-major head interleave) so the edge phase gathers 256B per edge. The tiny
attention projections (x @ W @ a_src/dst, ~50 MFLOP) are computed on host
and shipped as a pre-added per-edge-slot alpha table, which removes the
per-edge aS/aD gathers entirely. Bias-add + ReLU + head un-interleave run
on host between the two launches (b is folded there).

htab rows are partition-major (node v -> row (v%128)*NCH + v//128) so the
node-phase store and the xt load move 6KB-contiguous runs (full DMA rate);
the int16 gather-index split is then by partition half. The softmax division
runs on host from the raw [numerator | denominator] device output.

Per layer, per core:
  node phase:  h = xT_chunk.T @ Wperm (PE, bf16) -> htab rows (128 bf16),
               PSUM drained by DVE/Pool, stores on the Act HWDGE queue
  edge phase (per GS-tile group):
      dma_gather h rows by src (int16 idx; partition-half tables)
      Ind[e,d] = (iota == dstloc[e]) one-hot            (DVE/Pool, bf16)
      ex = exp(lrelu(alpha_pre)); h *= ex in place      (DVE+ACT, bf16)
      PSUM accum: out += Ind.T @ h_scaled, den += Ind.T @ ex   (PE, bf16)
      epilogue: raw [out|den] rows to DRAM (f32).
"""

import sys
import numpy as np
import ml_dtypes

sys.path.insert(0, "/opt/trn_rl_repo")

import concourse.bacc as bacc
import concourse.mybir as mybir
from concourse.bass_utils import run_bass_kernel_spmd
from concourse.tile import TileContext

f32 = mybir.dt.float32
bf16 = mybir.dt.bfloat16
i16 = mybir.dt.int16
i32 = mybir.dt.int32
npbf16 = ml_dtypes.bfloat16

import os as _os
P = 128
H = 4
C = 32
F = 128          # feature width (= H*C)
FA = F + H       # msgex width: h | ex
GS = int(_os.environ.get("GAT_GS", "7"))        # dst tiles per gather group
IND_SPLIT = int(_os.environ.get("GAT_IS", "9"))  # per 13 ind builds: n to DVE
MSGP = int(_os.environ.get("GAT_MSGP", "0"))     # msgex cols done on Pool

N_CORES = 8
N_NODES = 50000
N_EDGES = 800000

# device column order is c-major: dev col c*H+h <-> ref col h*C+c
PERM = np.arange(F).reshape(H, C).T.flatten()      # ref col for each dev col
IPERM = np.arange(F).reshape(C, H).T.flatten()     # dev col for each ref col

import os
_SKIP = set(os.environ.get("GAT_SKIP", "").split(","))  # perf-bisect flags


def _make_plan(src, dst, N, n_cores):
    npad = ((N + P * n_cores - 1) // (P * n_cores)) * (P * n_cores)
    npc = npad // n_cores
    NT = npc // P
    NCH = npad // P
    NLO = npad // 2                # nodes in the lo half table
    assert NLO <= 32767 and NLO % P == 0

    tile_of = dst // P             # global dst chunk id
    # htab rows are partition-major: node v -> row (v%P)*NCH + v//P, so the
    # node-phase store writes long contiguous runs. The int16 half split is
    # then by partition parity: partitions 0-63 = lo table, 64-127 = hi.
    src_hi = ((src % P) >= (P // 2)).astype(np.int64)

    cnt = np.zeros((NCH, 2), np.int64)
    np.add.at(cnt, (tile_of, src_hi), 1)
    cnt_ct = cnt.reshape(n_cores, NT, 2)
    # rank-match chunks across cores (each core sorts its own chunks by
    # degree) so the shared per-slot chunk count K = max over cores tracks
    # the mean instead of the max of unsorted counts; the host un-permutes
    # the output rows afterward
    asg = np.argsort(cnt_ct.sum(-1), axis=1, kind="stable")
    cnt_s = np.take_along_axis(cnt_ct, asg[:, :, None], axis=1)
    Klo = np.maximum(1, np.ceil(cnt_s[:, :, 0].max(axis=0) / P).astype(np.int64))
    Khi = np.ceil(cnt_s[:, :, 1].max(axis=0) / P).astype(np.int64)
    LOCH = int(Klo.sum())
    HICH = int(Khi.sum())
    TOTCH = LOCH + HICH
    CO_lo = np.concatenate([[0], np.cumsum(Klo)])[:-1]
    CO_hi = LOCH + np.concatenate([[0], np.cumsum(Khi)])[:-1]

    okey = tile_of * 2 + src_hi
    order = np.argsort(okey, kind="stable")
    s_src = src[order]
    s_dst = dst[order]
    s_key = okey[order]
    starts = np.searchsorted(s_key, np.arange(NCH * 2))
    ends = np.searchsorted(s_key, np.arange(NCH * 2) + 1)

    gsrci = np.zeros((n_cores, 16, 8 * TOTCH), np.int16)
    gloc = np.full((n_cores, P, TOTCH), float(P), np.float32)
    gsrcn = np.zeros((n_cores, P, TOTCH), np.int32)   # global src node (pads 0)
    gdstn = np.zeros((n_cores, P, TOTCH), np.int32)   # global dst node (pads 0)
    gpad = np.ones((n_cores, P, TOTCH), bool)

    for c in range(n_cores):
        for t in range(NT):
            g = c * NT + int(asg[c, t])
            for half, co, nk in ((0, CO_lo[t], Klo[t]), (1, CO_hi[t], Khi[t])):
                if nk == 0:
                    continue
                e0, e1 = starts[2 * g + half], ends[2 * g + half]
                n = e1 - e0
                npadn = int(nk) * P
                sv = np.zeros(npadn, np.int64)
                lv = np.full(npadn, P, np.int64)
                sn = np.zeros(npadn, np.int64)
                dn = np.zeros(npadn, np.int64)
                if n:
                    ev = s_src[e0:e1]
                    sv[:n] = ((ev % P) - (P // 2) * half) * NCH + ev // P
                    lv[:n] = s_dst[e0:e1] % P
                    sn[:n] = ev
                    dn[:n] = s_dst[e0:e1]
                j = np.arange(npadn)
                cc = 8 * int(co) + j // 16
                rr = j % 16
                gsrci[c, rr, cc] = sv
                kk = int(co) + j // P
                pp = j % P
                gloc[c, pp, kk] = lv
                gsrcn[c, pp, kk] = sn
                gdstn[c, pp, kk] = dn
                gpad[c, pp[:n], kk[:n]] = False

    gsrci = np.tile(gsrci, (1, 8, 1))

    groups = [(t0, min(t0 + GS, NT)) for t0 in range(0, NT, GS)]

    return dict(
        n_cores=n_cores, N=N, npad=npad, npc=npc, NT=NT, NCH=NCH, NLO=NLO,
        Klo=[int(k) for k in Klo], Khi=[int(k) for k in Khi],
        LOCH=LOCH, HICH=HICH, TOTCH=TOTCH,
        CO_lo=[int(o) for o in CO_lo], CO_hi=[int(o) for o in CO_hi],
        groups=groups, asg=asg,
        gsrci=gsrci, gloc=gloc, gsrcn=gsrcn, gdstn=gdstn, gpad=gpad,
    )


def _layer_inputs(plan, x, W, a_src, a_dst):
    """x: [npad, F] f32 (rows >= N zero). Returns per-core input maps."""
    npad, NCH, TOTCH = plan["npad"], plan["NCH"], plan["TOTCH"]
    W = np.asarray(W, np.float32)
    Ablk_s = np.zeros((F, H), np.float32)
    Ablk_d = np.zeros((F, H), np.float32)
    for h in range(H):
        Ablk_s[h * C:(h + 1) * C, h] = a_src[h]
        Ablk_d[h * C:(h + 1) * C, h] = a_dst[h]

    xt = np.ascontiguousarray(
        x.reshape(NCH, P, F).transpose(2, 0, 1)).astype(npbf16)
    wperm = W[:, PERM].astype(npbf16)

    aS = x @ (W @ Ablk_s)          # [npad, H] f32
    aD = x @ (W @ Ablk_d)
    alpha = aS[plan["gsrcn"]] + aD[plan["gdstn"]]   # [cores, P, TOTCH, H]
    alpha[plan["gpad"]] = 0.0
    alpha = alpha.astype(npbf16)

    return [
        dict(xt=xt, wcat=wperm, gsrci=plan["gsrci"][c],
             galpha=alpha[c], gloc=plan["gloc"][c])
        for c in range(plan["n_cores"])
    ]


def _build_layer_kernel(plan):
    NT, NCH, TOTCH, NLO = plan["NT"], plan["NCH"], plan["TOTCH"], plan["NLO"]
    Klo, Khi = plan["Klo"], plan["Khi"]
    CO_lo, CO_hi = plan["CO_lo"], plan["CO_hi"]
    npad = plan["npad"]

    nc = bacc.Bacc()
    xt = nc.dram_tensor("xt", [F, NCH, P], bf16, kind="ExternalInput")
    wcat = nc.dram_tensor("wcat", [F, F], bf16, kind="ExternalInput")
    gsrci = nc.dram_tensor("gsrci", [P, 8 * TOTCH], i16, kind="ExternalInput")
    galpha = nc.dram_tensor("galpha", [P, TOTCH, H], bf16, kind="ExternalInput")
    gloc = nc.dram_tensor("gloc", [P, TOTCH], f32, kind="ExternalInput")
    out = nc.dram_tensor("out", [NT * P, FA], f32, kind="ExternalOutput")

    htab = nc.dram_tensor("htab", [npad, F], bf16)

    # Phase 1: node phase (own TileContext; its exit barrier guarantees htab
    # is fully in DRAM before any edge-phase gather issues).
    with TileContext(nc) as tc:
        with (
            tc.tile_pool(name="const", bufs=1) as cpool,
            tc.tile_pool(name="nodein", bufs=4) as npool,
            tc.tile_pool(name="nodeout", bufs=4) as hpool,
            tc.tile_pool(name="npsum", bufs=4, space="PSUM") as npsum,
        ):
            wcat_sb = cpool.tile([F, F], bf16)
            nc.sync.dma_start(wcat_sb[:, :], wcat[:, :])

            NB = int(_os.environ.get("GAT_NB", "24"))
            node_batches = [] if "node" in _SKIP else [
                (b, min(NB, NCH - b)) for b in range(0, NCH, NB)
            ]
            cpy = 0
            for bi, (b, nb) in enumerate(node_batches):
                xcb = npool.tile([F, NB, P], bf16, tag="xc")
                nc.sync.dma_start(xcb[:, 0:nb, :], xt[:, b:b + nb, :])
                hcb = hpool.tile([P, NB, F], bf16, tag="hc")
                for k8 in range(0, nb, 8):
                    kk = min(8, nb - k8)
                    # 2-bank PSUM tile: each matmul stays inside a bank, the
                    # drain copy spans both (halves per-chunk init overhead)
                    ps = npsum.tile([P, 8 * F], f32, tag="nps")
                    for k in range(kk):
                        nc.tensor.matmul(
                            ps[:, k * F:(k + 1) * F], lhsT=xcb[:, k8 + k, :],
                            rhs=wcat_sb[:, :], start=True, stop=True)
                    dst_ap = hcb[:, k8:k8 + kk, :]
                    src_ap = ps[:, 0:kk * F].rearrange("p (k f) -> p k f", f=F)
                    # GPSIMD cannot access PSUM on trn2; drain on DVE with
                    # an occasional Act copy (Act mostly runs the htab queue)
                    if cpy % 4 == 3:
                        nc.scalar.copy(dst_ap, src_ap)
                    else:
                        nc.vector.tensor_copy(dst_ap, src_ap)
                    cpy += 1
                # htab store via the Act HWDGE queue (separate from xcb's SP);
                # partition-major row order -> 6KB contiguous runs
                nc.scalar.dma_start(
                    htab[:, :].rearrange("(p n) w -> p n w", p=P)[:, b:b + nb, :],
                    hcb[:, 0:nb, :])

    # Phase 2: edge phase.
    with TileContext(nc) as tc:
        with (
            tc.tile_pool(name="econst", bufs=1) as cpool,
            tc.tile_pool(name="egather", bufs=2) as gpool,
            tc.tile_pool(name="eex", bufs=2) as epool,
            tc.tile_pool(name="eind", bufs=2) as ipool,
            tc.tile_pool(name="epsum", bufs=8, space="PSUM") as epsum,
            tc.tile_pool(name="eout", bufs=3) as opool,
        ):
            iota_i = cpool.tile([P, P], i32)
            nc.gpsimd.iota(iota_i[:, :], pattern=[[1, P]], base=0,
                           channel_multiplier=0)
            iota_f = cpool.tile([P, P], bf16)
            nc.vector.tensor_copy(iota_f[:, :], iota_i[:, :])

            srcA = cpool.tile([P, 8 * TOTCH], i16)
            nc.sync.dma_start(srcA[:, :], gsrci[:, :])
            locA = cpool.tile([P, TOTCH], f32)
            nc.scalar.dma_start(locA[:, :], gloc[:, :])
            galA = cpool.tile([P, TOTCH, H], bf16)
            nc.scalar.dma_start(galA[:, :, :], galpha[:, :, :])

            indcnt = 0
            for g0, g1 in plan["groups"]:
                if "edge" in _SKIP:
                    break
                clo0 = CO_lo[g0]
                clo1 = CO_lo[g1 - 1] + Klo[g1 - 1]
                chi0 = CO_hi[g0]
                chi1 = CO_hi[g1 - 1] + Khi[g1 - 1]
                nlo, nhi = clo1 - clo0, chi1 - chi0
                ng = nlo + nhi

                halves = []
                hsa_lo = gpool.tile([P, nlo, F], bf16, tag="hlo")
                nc.gpsimd.dma_gather(
                    out_ap=hsa_lo[:, :, :], in_ap=htab[0:NLO, :],
                    idxs_ap=srcA[:, 8 * clo0:8 * clo1],
                    num_idxs=nlo * P, num_idxs_reg=nlo * P, elem_size=F,
                    single_packet=False)
                halves.append((hsa_lo, clo0, nlo))
                if nhi > 0:
                    hsa_hi = gpool.tile([P, nhi, F], bf16, tag="hhi")
                    nc.gpsimd.dma_gather(
                        out_ap=hsa_hi[:, :, :], in_ap=htab[NLO:npad, :],
                        idxs_ap=srcA[:, 8 * chi0:8 * chi1],
                        num_idxs=nhi * P, num_idxs_reg=nhi * P, elem_size=F,
                        single_packet=False)
                    halves.append((hsa_hi, chi0, nhi))

                # one-hot builds for the whole group: no data deps on the
                # gathers, so they fill the gather latency on DVE/Pool
                indg = ipool.tile([P, ng, P], bf16, tag="ind")
                for j in range(ng):
                    co = (clo0 + j) if j < nlo else (chi0 + j - nlo)
                    eng = (nc.vector if indcnt % 13 < IND_SPLIT
                           else nc.gpsimd)
                    eng.tensor_scalar(
                        out=indg[:, j, :], in0=iota_f[:, :],
                        scalar1=locA[:, co:co + 1],
                        scalar2=None, op0=mybir.AluOpType.is_equal)
                    indcnt += 1

                msgs = []
                for hsa, c0, nch in halves:
                    lrl = epool.tile([P, nch, H], bf16, tag="lrl")
                    nc.vector.scalar_tensor_tensor(
                        out=lrl[:, :, :], in0=galA[:, c0:c0 + nch, :],
                        scalar=0.2, in1=galA[:, c0:c0 + nch, :],
                        op0=mybir.AluOpType.mult, op1=mybir.AluOpType.max)
                    ex = epool.tile([P, nch, H], bf16, tag="ex")
                    nc.scalar.activation(ex[:, :, :], lrl[:, :, :],
                                         mybir.ActivationFunctionType.Exp)
                    # scale the gathered h by ex in place (no msg buffer);
                    # optionally give Pool a column share (MSGP of 128)
                    cd = F - MSGP
                    nc.vector.tensor_tensor(
                        out=hsa[:, :, 0:cd].rearrange(
                            "p k (c h) -> p k c h", h=H),
                        in0=hsa[:, :, 0:cd].rearrange(
                            "p k (c h) -> p k c h", h=H),
                        in1=ex[:, :, :].rearrange(
                            "p k (o h) -> p k o h", o=1).to_broadcast(
                            [P, nch, cd // H, H]),
                        op=mybir.AluOpType.mult)
                    if MSGP:
                        nc.gpsimd.tensor_tensor(
                            out=hsa[:, :, cd:F].rearrange(
                                "p k (c h) -> p k c h", h=H),
                            in0=hsa[:, :, cd:F].rearrange(
                                "p k (c h) -> p k c h", h=H),
                            in1=ex[:, :, :].rearrange(
                                "p k (o h) -> p k o h", o=1).to_broadcast(
                                [P, nch, MSGP // H, H]),
                            op=mybir.AluOpType.mult)
                    msgs.append((hsa, ex, c0, nch))

                for t in range(g0, g1):
                    pso = epsum.tile([P, FA], f32, tag="pso")
                    nk = Klo[t] + Khi[t]
                    ki = 0
                    for hv, (m, ex, c0, nch) in enumerate(msgs):
                        co = CO_lo[t] if hv == 0 else CO_hi[t]
                        cnt = Klo[t] if hv == 0 else Khi[t]
                        off = co - c0
                        goff = (co - clo0) if hv == 0 else (nlo + co - chi0)
                        for k in range(cnt):
                            # start pending-zeroes the whole 2KB psum zero
                            # region, so only the first matmul of the tile
                            # starts and only the last one stops
                            nc.tensor.matmul(
                                pso[:, 0:F], lhsT=indg[:, goff + k, :],
                                rhs=m[:, off + k, :],
                                start=(ki == 0), stop=False,
                                skip_group_check=(ki != 0))
                            nc.tensor.matmul(
                                pso[:, F:FA], lhsT=indg[:, goff + k, :],
                                rhs=ex[:, off + k, :],
                                start=False, stop=(ki == nk - 1),
                                skip_group_check=(ki != nk - 1))
                            ki += 1

                    on = opool.tile([P, FA], f32, tag="on")
                    nc.scalar.copy(on[:, :], pso[:, :])
                    nc.sync.dma_start(out[t * P:(t + 1) * P, :], on[:, :])
            if "edge" in _SKIP:
                zo = cpool.tile([P, FA], f32)
                nc.vector.memset(zo[:, :], 0.0)
                for t in range(NT):
                    nc.sync.dma_start(out[t * P:(t + 1) * P, :], zo[:, :])

    nc.finalize()
    return nc


_KERNEL_CACHE = {}


def _get_kernel(plan):
    key = (tuple(plan["Klo"]), tuple(plan["Khi"]), plan["npad"])
    if key not in _KERNEL_CACHE:
        _KERNEL_CACHE[key] = _build_layer_kernel(plan)
    return _KERNEL_CACHE[key]


def _run_layer(nc, maps, trace=False):
    last = None
    for attempt in range(3):
        try:
            res = run_bass_kernel_spmd(nc, maps, list(range(len(maps))),
                                       trace=trace)
            outs = [r["out"] for r in res.results]
            return np.concatenate(outs, axis=0), res
        except Exception as e:  # transient NRT_EXEC_UNIT_UNRECOVERABLE etc.
            last = e
            import time as _time
            _time.sleep(2.0 * (attempt + 1))
    raise last


def _normalize(o, plan):
    """Undo the rank-match row permutation, then num/den ((c,h) cols)."""
    NT, NCH = plan["NT"], plan["NCH"]
    idx = (np.arange(plan["n_cores"])[:, None] * NT + plan["asg"]).ravel()
    of = np.empty_like(o).reshape(NCH, P, FA)
    of[idx] = o.reshape(NCH, P, FA)
    of = of.reshape(-1, FA)
    num = of[:, 0:F].reshape(-1, C, H)
    den = of[:, F:FA]
    return (num / (den[:, None, :] + 1e-16)).reshape(-1, F)


def kernel(x, edge_index, W1, a_src1, a_dst1, b1, W2, a_src2, a_dst2, b2,
           _trace=False, _collect=None):
    x = np.asarray(x, dtype=np.float32)
    edge_index = np.asarray(edge_index)
    assert x.shape == (N_NODES, F), x.shape
    assert edge_index.shape == (2, N_EDGES), edge_index.shape

    loops = np.arange(N_NODES, dtype=np.int64)
    src = np.concatenate([edge_index[0].astype(np.int64), loops])
    dst = np.concatenate([edge_index[1].astype(np.int64), loops])

    plan = _make_plan(src, dst, N_NODES, N_CORES)
    nc = _get_kernel(plan)
    npad = plan["npad"]

    xp = np.zeros((npad, F), np.float32)
    xp[:N_NODES] = x
    maps1 = _layer_inputs(plan, xp, np.asarray(W1), np.asarray(a_src1),
                          np.asarray(a_dst1))
    o1, res1 = _run_layer(nc, maps1, trace=_trace)
    o1 = _normalize(o1, plan)

    h1 = np.maximum(o1[:, IPERM] + np.asarray(b1, np.float32), 0.0)
    h1[N_NODES:] = 0.0
    maps2 = _layer_inputs(plan, h1, np.asarray(W2), np.asarray(a_src2),
                          np.asarray(a_dst2))
    o2, res2 = _run_layer(nc, maps2, trace=_trace)
    o2 = _normalize(o2, plan)

    if _collect is not None:
        _collect.extend([res1, res2])
    return np.maximum(o2[:N_NODES][:, IPERM] + np.asarray(b2, np.float32),
                      0.0).astype(np.float32)

